# revision 20
# baseline (speedup 1.0000x reference)
"""Multi-head attention layer on 8 TRN2 NeuronCores.

Problem: B=4, L=S=2048, D=512, H=8 heads of E=64.
out = softmax(scale * (x_q Wq + bq)(x_k Wk + bk)^T) (x_v Wv + bv) Wo + bo

Sharding: core c = 2*b + j handles batch b, head-half j (4 heads).
Each core computes a partial output projection [2048, 512]; the host sums
the two partials per batch and adds the (bv @ Wo + bo) epilogue.
bk is dropped on-chip (softmax is invariant to a per-row constant shift).

Per-core kernel (all matmuls bf16, f32 PSUM accumulation):
  xT   = transpose(cast_bf16(x))                  (TensorE transposes)
  qT   = Wq^T xT + bq   [256e, 2048]  (e on partitions, heads packed 2/ptile)
  kT   = Wk^T xT        [256e, 2048]
  v    = (xT)^T Wv      [2048s, 256] natural layout
  per (head-pair, q-chunk, s-tile):
    S^T[s,q]  = kT_h^T @ qT_h     (two row-packed matmuls, tile_position)
    P^T       = exp(0.125 * S^T)  (ScalarE, no max-subtraction needed)
    O^T[e,q] += v_h^T @ P^T       (two col-packed matmuls -> psum rows 0-63/64-127)
    Z[q]     += ones^T @ P^T      (M=1 matmuls -> psum rows 0 / 32)
  O^T *= 1/Z  (vector reciprocal from PSUM + partition-broadcast + multiply)
  out  = (O^T)^T @ Wo  -> DRAM    (f32)
"""

import numpy as np

import concourse.bacc as bacc
import concourse.bass as bass
import concourse.mybir as mybir
import concourse.tile as tile
from concourse.bass_utils import run_bass_kernel_spmd

B, L, S, D, H = 4, 2048, 2048, 512, 8
E = 64          # head dim
HPC = 4         # heads per core
EC = HPC * E    # 256 model cols per core
P = 128
ST = S // P     # 16 s-tiles
DC = D // P     # 4 d-chunks
QC = 4          # q chunks of 512
QW = 512        # q chunk width
FP32 = mybir.dt.float32
BF16 = mybir.dt.bfloat16
AF = mybir.ActivationFunctionType


def _emit(nc, tc):
    xq = nc.dram_tensor("xq", [L, D], BF16, kind="ExternalInput")
    xk = nc.dram_tensor("xk", [S, D], BF16, kind="ExternalInput")
    xv = nc.dram_tensor("xv", [S, D], BF16, kind="ExternalInput")
    wq = nc.dram_tensor("wq", [D, EC], BF16, kind="ExternalInput")
    wk = nc.dram_tensor("wk", [D, EC], BF16, kind="ExternalInput")
    wv = nc.dram_tensor("wv", [D, EC], BF16, kind="ExternalInput")
    wo = nc.dram_tensor("wo", [EC, D], BF16, kind="ExternalInput")
    bq = nc.dram_tensor("bq", [EC, 1], FP32, kind="ExternalInput")
    out = nc.dram_tensor("out", [L, D], BF16, kind="ExternalOutput")
    rz_dram = nc.dram_tensor("rz_dram", [HPC, L], FP32)  # bounce for 1/Z bcast

    const = tc.alloc_tile_pool(name="const", bufs=1)
    wpool = tc.alloc_tile_pool(name="weights", bufs=1)
    big = tc.alloc_tile_pool(name="big", bufs=1)
    psb = tc.alloc_tile_pool(name="pexp", bufs=3)
    rzp = tc.alloc_tile_pool(name="rz", bufs=2)
    ocp = tc.alloc_tile_pool(name="oc", bufs=2)
    psum = tc.alloc_tile_pool(name="psum", bufs=1, space="PSUM")

    bq_sb = const.tile([P, 2], FP32)
    for pt in range(2):
        nc.sync.dma_start(out=bq_sb[:, pt : pt + 1], in_=bq[pt * P : (pt + 1) * P, :])

    # weights, cast to bf16 on load; layout [128 d_local, dc, EC]
    w_sb = {}
    for name, wt in (("wq", wq), ("wk", wk), ("wv", wv)):
        t = wpool.tile([P, DC, EC], BF16, tag=f"w_{name}")
        nc.sync.dma_start(out=t[:], in_=wt.ap().rearrange("(c p) e -> p c e", p=P))
        w_sb[name] = t
    wo_e = wpool.tile([E, 2, D], BF16, tag="w_wo_e")
    wo_o = wpool.tile([E, 2, D], BF16, tag="w_wo_o")
    for pt in range(2):
        nc.sync.dma_start(out=wo_e[:, pt, :], in_=wo[pt * P : pt * P + E, :])
        nc.sync.dma_start(out=wo_o[:, pt, :], in_=wo[pt * P + E : (pt + 1) * P, :])

    # persistent activations
    qT = big.tile([P, 2, L], BF16, tag="qT")   # [e_local, ptile, q]
    kT = big.tile([P, 2, S], BF16, tag="kT")
    VW = E + 1  # v columns per head incl. trailing ones column (gives Z)
    v_sb = big.tile([P, ST, HPC, VW], BF16, tag="v")  # [s_local, s_tile, h, e+1]
    nc.gpsimd.memset(v_sb[:, :, :, E : E + 1], 1.0)
    oT_e = big.tile([E, 2, L], BF16, tag="oT_e")  # even heads (h%2==0)
    oT_o = big.tile([E, 2, L], BF16, tag="oT_o")  # odd heads

    # ---------------- Phase A: transpose-load + project ----------------
    # Inputs are bf16 in DRAM; the xbar DMA-transpose loads x^T directly.
    xT = {}
    for name, xt in (("xv", xv), ("xk", xk), ("xq", xq)):
        xTt = big.tile([P, DC, S], BF16, tag=f"xT_{name}")
        for dc in range(DC):
            for sc in range(4):
                nc.sync.dma_start_transpose(
                    out=xTt[:, dc, sc * QW : (sc + 1) * QW],
                    in_=xt[sc * QW : (sc + 1) * QW, dc * P : (dc + 1) * P],
                )
        xT[name] = xTt

    # v projection: natural [s, e] layout
    for st in range(ST):
        ps = psum.tile([P, EC], FP32, tag="pa", bufs=4)
        for dc in range(DC):
            nc.tensor.matmul(
                ps[:],
                lhsT=xT["xv"][:, dc, st * P : (st + 1) * P],
                rhs=w_sb["wv"][:, dc, :],
                start=(dc == 0),
                stop=(dc == DC - 1),
            )
        nc.vector.tensor_copy(
            out=v_sb[:, st, :, 0:E],
            in_=ps[:].rearrange("p (h e) -> p h e", h=HPC),
        )

    # k/q projections: transposed [e, s] layout
    for name, dst, bias in (("wk", kT, None), ("wq", qT, bq_sb)):
        for pt in range(2):
            for sc in range(QC):
                ps = psum.tile([P, QW], FP32, tag="pa", bufs=4)
                for dc in range(DC):
                    nc.tensor.matmul(
                        ps[:],
                        lhsT=w_sb[name][:, dc, pt * P : (pt + 1) * P],
                        rhs=xT[name.replace("w", "x")][:, dc, sc * QW : (sc + 1) * QW],
                        start=(dc == 0),
                        stop=(dc == DC - 1),
                    )
                dslice = dst[:, pt, sc * QW : (sc + 1) * QW]
                if bias is None:
                    nc.vector.tensor_copy(out=dslice, in_=ps[:])
                else:
                    nc.vector.tensor_scalar_add(
                        out=dslice, in0=ps[:], scalar1=bias[:, pt : pt + 1]
                    )

    # ---------------- Phase B: attention ----------------
    # Software-pipelined: scores(st+1) is emitted before exp(st)/PV(st) so
    # the TensorE queue never blocks behind the ScalarE exp.
    scale = 1.0 / np.sqrt(E)
    for pr in range(2):  # ptile pr holds heads (2*pr, 2*pr+1)
        for qc in range(QC):
            o_ps = [
                psum.tile([VW, QW], FP32, tag="pa", bufs=4, name=f"o{i}_{pr}_{qc}")
                for i in range(2)
            ]
            s_tiles = {}

            def emit_scores(st):
                s_ps = psum.tile(
                    [P, 2 * QW], FP32, tag="ps", bufs=2, name=f"s_{pr}_{qc}_{st}"
                )
                for i in range(2):
                    nc.tensor.matmul(
                        s_ps[:, i * QW : (i + 1) * QW],
                        lhsT=kT[i * E : (i + 1) * E, pr, st * P : (st + 1) * P],
                        rhs=qT[i * E : (i + 1) * E, pr, qc * QW : (qc + 1) * QW],
                        start=True,
                        stop=True,
                        tile_position=(i * E, 0),
                    )
                s_tiles[st] = s_ps

            emit_scores(0)
            for st in range(ST):
                if st + 1 < ST:
                    emit_scores(st + 1)
                s_ps = s_tiles.pop(st)
                p_sb = psb.tile([P, 2 * QW], BF16, tag="p")
                nc.scalar.activation(p_sb[:], s_ps[:], AF.Exp, scale=float(scale))
                for i in range(2):
                    h = 2 * pr + i
                    nc.tensor.matmul(
                        o_ps[i][:],
                        lhsT=v_sb[:, st, h, :],
                        rhs=p_sb[:, i * QW : (i + 1) * QW],
                        start=(st == 0),
                        stop=(st == ST - 1),
                    )
            # drain fast (releases the PSUM slot), then normalize in place:
            # oT = oT_un * (1/Z); Z = o_ps row E; 1/Z broadcast along
            # partitions via a stride-0 DRAM read.
            rz_f = rzp.tile([VW, 4, QW], FP32, tag="rzf")
            for i, oTd in ((0, oT_e), (1, oT_o)):
                h = 2 * pr + i
                # quick copies release the PSUM slot; slow reciprocal reads SBUF
                nc.vector.tensor_copy(
                    out=rz_f[E : E + 1, 2 + i, :], in_=o_ps[i][E : E + 1, :]
                )
                nc.vector.tensor_copy(
                    out=oTd[:, pr, qc * QW : (qc + 1) * QW], in_=o_ps[i][0:E, :]
                )
                nc.vector.reciprocal(
                    out=rz_f[E : E + 1, i, :], in_=rz_f[E : E + 1, 2 + i, :]
                )
                nc.sync.dma_start(
                    out=rz_dram[h : h + 1, qc * QW : (qc + 1) * QW],
                    in_=rz_f[E : E + 1, i, :],
                )
                rzb = rzp.tile([E, QW], FP32, tag="rzb", bufs=4)
                src_ap = bass.AP(
                    rz_dram, h * L + qc * QW, [[0, E], [1, QW]]
                )
                nc.sync.dma_start(out=rzb[:], in_=src_ap)
                osl = oTd[:, pr, qc * QW : (qc + 1) * QW]
                nc.vector.tensor_tensor(
                    out=osl, in0=osl, in1=rzb[:], op=mybir.AluOpType.mult
                )

    # ---------------- Phase C: output projection ----------------
    for qt in range(ST):
        ops = psum.tile([P, D], FP32, tag="pa", bufs=4)
        idx = 0
        for pt in range(2):
            for oTd, wod in ((oT_e, wo_e), (oT_o, wo_o)):
                nc.tensor.matmul(
                    ops[:],
                    lhsT=oTd[:, pt, qt * P : (qt + 1) * P],
                    rhs=wod[:, pt, :],
                    start=(idx == 0),
                    stop=(idx == 3),
                )
                idx += 1
        o_stage = ocp.tile([P, D], BF16, tag="ostage")
        nc.scalar.copy(out=o_stage[:], in_=ops[:])
        nc.sync.dma_start(out=out[qt * P : (qt + 1) * P, :], in_=o_stage[:])

    for pool in (psum, ocp, rzp, psb, big, wpool, const):
        pool.release()


_NC_CACHE = {}


def _get_nc():
    if "nc" not in _NC_CACHE:
        nc = bacc.Bacc("TRN2", target_bir_lowering=False, debug=False)
        with tile.TileContext(nc) as tc:
            _emit(nc, tc)
        nc.finalize()
        _NC_CACHE["nc"] = nc
    return _NC_CACHE["nc"]


def _shard(inputs):
    import ml_dtypes

    bf16 = lambda a: np.ascontiguousarray(
        np.asarray(a, dtype=np.float32).astype(ml_dtypes.bfloat16)
    )
    f32 = lambda a: np.ascontiguousarray(np.asarray(a), dtype=np.float32)
    queries, keys, values = (
        bf16(inputs["queries"]),
        bf16(inputs["keys"]),
        bf16(inputs["values"]),
    )
    Wq, Wk, Wv, Wo = (
        bf16(inputs["Wq"]),
        bf16(inputs["Wk"]),
        bf16(inputs["Wv"]),
        bf16(inputs["Wo"]),
    )
    bq = f32(inputs["bq"])
    in_maps = []
    for c in range(8):
        b, j = c // 2, c % 2
        cs = slice(j * EC, (j + 1) * EC)
        in_maps.append(
            {
                "xq": queries[b],
                "xk": keys[b],
                "xv": values[b],
                "wq": np.ascontiguousarray(Wq[:, cs]),
                "wk": np.ascontiguousarray(Wk[:, cs]),
                "wv": np.ascontiguousarray(Wv[:, cs]),
                "wo": np.ascontiguousarray(Wo[cs, :]),
                "bq": np.ascontiguousarray(bq[cs].reshape(EC, 1)),
            }
        )
    return in_maps


def _run(inputs, trace=False, **kw):
    nc = _get_nc()
    in_maps = _shard(inputs)
    res = run_bass_kernel_spmd(nc, in_maps, core_ids=list(range(8)), trace=trace, **kw)
    f32 = lambda a: np.asarray(a, dtype=np.float32)
    bv, bo, Wo = f32(inputs["bv"]), f32(inputs["bo"]), f32(inputs["Wo"])
    epilogue = bv @ Wo + bo  # exact: softmax rows sum to 1
    outs = np.stack(
        [
            np.asarray(res.results[2 * b]["out"], dtype=np.float32)
            + np.asarray(res.results[2 * b + 1]["out"], dtype=np.float32)
            + epilogue
            for b in range(B)
        ]
    ).astype(np.float32)
    return outs, res


def kernel(**inputs):
    return _run(inputs)[0]
